# revision 3
# baseline (speedup 1.0000x reference)
"""Trainium2 Bass kernel for nn_ClosedArap (ARAP rhs, GNN message passing), v3.

rhs_i = sum_k w_ik * 0.5 * (R_i + R_j) @ (p_i - p_j),  j = nbr[i, k]
      = R_i @ (sum_k w'_ik d_ik) + sum_k R_j @ (w'_ik d_ik),   w' = w/2

The axon link to the device runs at ~40 MB/s up / ~23 MB/s down, so a
device invocation's cost is dominated by uploaded bytes.  This version
ships the minimum that keeps a comfortable accuracy margin:
  - per-vertex table rows T[v] = [p_v 3x f16 | R_v 9x f16] (24B); only a
    1/8 shard is uploaded per core and an on-device AllGather replicates
    the full table into each core's DRAM,
  - neighbor ids packed to 20 bits (2.5B/edge: two byte planes + a
    nibble plane), unpacked on DVE with shifts/ors,
  - weights quantized to u8 (w' = u8/510, descaled at load; rhs is
    linear in w so this costs no relative accuracy at the output),
  - local rows streamed from the core's own shard (no upload),
  - rhs downloaded as f16.
Total rel err vs the f32 reference ~1.6e-3 (threshold 2e-2); an int8
rotation variant (16B rows) saved another 8 MB but measured 1.15e-2 --
too close to the gate.

Gathers are one-offset-per-partition indirect DMAs (the multi-offset
"vector dynamic offset" DMA form generates descriptors for only one
partition in this stack), rotated over 4 SWDGE queues so the per-queue
16-bit DMA-semaphore fields stay under 65535 across the single-NEFF
invocation (8192/4 * 16 = 32768).

Slot map: core c owns vertex slots [c*SLOTS, (c+1)*SLOTS), slot (p, t)
of core c holds vertex c*SLOTS + p*NT + t; host staging is pure reshape
of [NPAD]-padded arrays.
"""
import numpy as np

from concourse import bass, bacc, mybir, tile

N_FULL = 1_000_000
K = 8
NCORES = 8
D = 12            # packed row: p(3) + R(9), f16
GRP = 16          # vertex tiles (of 128) per pipeline group
NT = 992          # vertex tiles per core; 128*992 = 126976 slots
SLOTS = 128 * NT
NPAD = NCORES * SLOTS          # 1015808 padded table rows (< 2^20)
NQ = 4            # SWDGE queues to rotate gathers over
SW = 510.0        # weight u8 scale (w' = w/2 = u8/510)
OPB = 5 * K // 2  # offset bytes per vertex slot (2.5 per edge)
T_B = SLOTS * D * 2                    # table region bytes
O_B = 128 * NT * OPB                   # offset region bytes
W_B = 128 * NT * K                     # weight region bytes
BLOB_B = T_B + O_B + W_B               # fused input bytes per core

LAST_EXEC_NS = None
LAST_RUN_WALL_S = None

_CACHE = {}


def build_kernel():
    ngrp = NT // GRP
    ek = GRP * K                  # neighbor slots per partition per group
    ob_w = 5 * ek // 2            # offset bytes per partition per group
    nc = bacc.Bacc("TRN2", target_bir_lowering=False, debug=False,
                   num_devices=NCORES, num_swdge_queues=NQ)
    f16 = mybir.dt.float16
    f32 = mybir.dt.float32
    u8 = mybir.dt.uint8
    i32 = mybir.dt.int32
    # one fused input blob per core: [table f16 rows | offset planes | w u8]
    blob = nc.dram_tensor("blob", [BLOB_B], u8, kind="ExternalInput").ap()
    tsh16 = blob.bitcast(f16)           # f16 view for the table region
    rhs = nc.dram_tensor("rhs", [128, NT * 3], f16, kind="ExternalOutput").ap()

    with tile.TileContext(nc) as tc:
        with tc.tile_pool(name="dram", bufs=1, space="DRAM") as dpool, \
                tc.tile_pool(name="sbuf", bufs=3) as pool:
            bounce = dpool.tile([SLOTS, D], f16, tag="bounce")
            table = dpool.tile([NPAD, D], f16, tag="table")
            nc.sync.dma_start(
                out=bounce[:],
                in_=bass.AP(tsh16.tensor, 0, [(D, SLOTS), (1, D)]))
            nc.gpsimd.collective_compute(
                "AllGather", mybir.AluOpType.bypass,
                replica_groups=[list(range(NCORES))],
                ins=[bounce.opt()], outs=[table.opt()])

            table_ap = table[:]
            bounce_t = bounce.tensor
            bounce_off = bounce[:].offset
            qi = 0

            for g in range(ngrp):
                ob_t = pool.tile([128, ob_w], u8, tag="offb")
                o_t = pool.tile([128, ek], i32, tag="off")
                ot_t = pool.tile([128, ek], i32, tag="offt")
                on_t = pool.tile([128, ek // 2], i32, tag="offn")
                w8_t = pool.tile([128, ek], u8, tag="wgt8")
                w_t = pool.tile([128, ek], f16, tag="wgt")
                pr_t = pool.tile([128, GRP, D], f16, tag="locpr")
                g_t = pool.tile([128, ek, D], f16, tag="gath")
                df_t = pool.tile([128, ek, 3], f16, tag="diff")
                m_t = pool.tile([128, ek, 3], f16, tag="macc")
                u_t = pool.tile([128, ek, 3], f16, tag="utmp")
                t2_t = pool.tile([128, GRP, 3], f32, tag="t2r")
                sv_t = pool.tile([128, GRP, 3], f32, tag="svr")
                rf_t = pool.tile([128, GRP, 9], f32, tag="rif")
                u2_t = pool.tile([128, GRP, 3], f32, tag="u2")
                o2_t = pool.tile([128, GRP, 3], f32, tag="out")
                o16_t = pool.tile([128, GRP, 3], f16, tag="out16")

                nc.sync.dma_start(
                    out=ob_t[:],
                    in_=bass.AP(blob.tensor, T_B + g * ob_w,
                                [(NT * OPB, 128), (1, ob_w)]))
                nc.sync.dma_start(
                    out=w8_t[:],
                    in_=bass.AP(blob.tensor, T_B + O_B + g * ek,
                                [(NT * K, 128), (1, ek)]))
                # local rows: bounce[p*NT + g*GRP + t] for t in [0,GRP)
                loc = bass.AP(bounce_t, bounce_off + g * GRP * D,
                              [(NT * D, 128), (1, GRP * D)])
                nc.sync.dma_start(out=pr_t[:], in_=loc)

                # ids = b0 | (b1 << 8) | (nibble << 16); bit ops can't
                # cast, so cast-copy each plane to i32 first
                nc.vector.tensor_copy(out=o_t[:], in_=ob_t[:, 0:ek])
                nc.vector.tensor_copy(out=ot_t[:], in_=ob_t[:, ek:2 * ek])
                nc.vector.tensor_scalar(
                    out=ot_t[:], in0=ot_t[:], scalar1=8, scalar2=None,
                    op0=mybir.AluOpType.logical_shift_left)
                nc.vector.tensor_tensor(out=o_t[:], in0=o_t[:], in1=ot_t[:],
                                        op=mybir.AluOpType.bitwise_or)
                nc.vector.tensor_copy(
                    out=on_t[:], in_=ob_t[:, 2 * ek:2 * ek + ek // 2])

                def o_half(par):
                    return bass.AP(o_t.tensor, o_t[:].offset + par,
                                   [o_t[:].ap[0], (2, ek // 2)])
                # even edges: low nibble; odd edges: high nibble
                nc.vector.tensor_scalar(
                    out=ot_t[:, 0:ek // 2], in0=on_t[:], scalar1=15,
                    scalar2=16, op0=mybir.AluOpType.bitwise_and,
                    op1=mybir.AluOpType.logical_shift_left)
                nc.vector.tensor_tensor(
                    out=o_half(0), in0=o_half(0), in1=ot_t[:, 0:ek // 2],
                    op=mybir.AluOpType.bitwise_or)
                nc.vector.tensor_scalar(
                    out=ot_t[:, 0:ek // 2], in0=on_t[:], scalar1=4,
                    scalar2=16, op0=mybir.AluOpType.logical_shift_right,
                    op1=mybir.AluOpType.logical_shift_left)
                nc.vector.tensor_tensor(
                    out=o_half(1), in0=o_half(1), in1=ot_t[:, 0:ek // 2],
                    op=mybir.AluOpType.bitwise_or)

                # w' = u8 / 510  (cast copy, then scale in f16)
                nc.vector.tensor_copy(out=w_t[:], in_=w8_t[:])
                nc.vector.tensor_scalar(
                    out=w_t[:], in0=w_t[:], scalar1=1.0 / SW, scalar2=None,
                    op0=mybir.AluOpType.mult)

                # neighbor rows: one offset per partition per instruction,
                # rotated across the 4 SWDGE queues
                for c in range(ek):
                    bi = nc.gpsimd.indirect_dma_start(
                        out=g_t[:, c, :], out_offset=None, in_=table_ap,
                        in_offset=bass.IndirectOffsetOnAxis(
                            ap=o_t[:, c:c + 1], axis=0))
                    if qi % NQ:
                        bi.ins.queue = f"qPoolDynamic{qi % NQ}"
                    qi += 1

                gp = g_t[:, :, 0:3]      # gathered p_j  [128, ek, 3]
                prp = bass.AP(pr_t.tensor, pr_t[:].offset,
                              [pr_t[:].ap[0], (D, GRP), (0, K), (1, 3)])

                def gr_col(c):
                    # gathered R_j rows a, column c -> [128, ek, 3]
                    return bass.AP(g_t.tensor, g_t[:].offset + 3 + c,
                                   [g_t[:].ap[0], (D, ek), (3, 3)])

                def df_col(c):
                    # wd[:, e, c] broadcast over a -> [128, ek, 3]
                    return bass.AP(df_t.tensor, df_t[:].offset + c,
                                   [df_t[:].ap[0], (3, ek), (0, 3)])

                # wd = (p_i - p_j) * w'   (in place on df_t)
                nc.vector.tensor_tensor(out=df_t[:], in0=prp,
                                        in1=gp, op=mybir.AluOpType.subtract)
                wv = bass.AP(w_t.tensor, w_t[:].offset,
                             [w_t[:].ap[0], (1, ek), (0, 3)])
                nc.vector.tensor_tensor(out=df_t[:], in0=df_t[:], in1=wv,
                                        op=mybir.AluOpType.mult)

                # m = R_j @ wd  (per edge; R in raw i8 units)
                nc.vector.tensor_tensor(out=m_t[:], in0=gr_col(0),
                                        in1=df_col(0), op=mybir.AluOpType.mult)
                nc.vector.tensor_tensor(out=u_t[:], in0=gr_col(1),
                                        in1=df_col(1), op=mybir.AluOpType.mult)
                nc.vector.tensor_tensor(out=m_t[:], in0=m_t[:], in1=u_t[:],
                                        op=mybir.AluOpType.add)
                nc.vector.tensor_tensor(out=u_t[:], in0=gr_col(2),
                                        in1=df_col(2), op=mybir.AluOpType.mult)
                nc.vector.tensor_tensor(out=m_t[:], in0=m_t[:], in1=u_t[:],
                                        op=mybir.AluOpType.add)

                # reduce over k: t2 = sum_k m, sv = sum_k wd   (f32 accum)
                def red_view(t):
                    return bass.AP(t.tensor, t[:].offset,
                                   [t[:].ap[0], (3 * K, GRP), (1, 3), (3, K)])
                nc.vector.tensor_reduce(out=t2_t[:], in_=red_view(m_t),
                                        axis=mybir.AxisListType.X,
                                        op=mybir.AluOpType.add)
                nc.vector.tensor_reduce(out=sv_t[:], in_=red_view(df_t),
                                        axis=mybir.AxisListType.X,
                                        op=mybir.AluOpType.add)

                # term1 = R_i @ sv  (f32, per vertex; R_i f16 -> f32)
                nc.vector.tensor_copy(
                    out=rf_t[:],
                    in_=bass.AP(pr_t.tensor, pr_t[:].offset + 3,
                                [pr_t[:].ap[0], (D, GRP), (1, 9)]))

                def rf_col(c):
                    return bass.AP(rf_t.tensor, rf_t[:].offset + c,
                                   [rf_t[:].ap[0], (9, GRP), (3, 3)])

                def sv_col(c):
                    return bass.AP(sv_t.tensor, sv_t[:].offset + c,
                                   [sv_t[:].ap[0], (3, GRP), (0, 3)])

                nc.vector.tensor_tensor(out=o2_t[:], in0=rf_col(0),
                                        in1=sv_col(0), op=mybir.AluOpType.mult)
                nc.vector.tensor_tensor(out=u2_t[:], in0=rf_col(1),
                                        in1=sv_col(1), op=mybir.AluOpType.mult)
                nc.vector.tensor_tensor(out=o2_t[:], in0=o2_t[:], in1=u2_t[:],
                                        op=mybir.AluOpType.add)
                nc.vector.tensor_tensor(out=u2_t[:], in0=rf_col(2),
                                        in1=sv_col(2), op=mybir.AluOpType.mult)
                nc.vector.tensor_tensor(out=o2_t[:], in0=o2_t[:], in1=u2_t[:],
                                        op=mybir.AluOpType.add)
                nc.vector.tensor_tensor(out=o2_t[:], in0=o2_t[:], in1=t2_t[:],
                                        op=mybir.AluOpType.add)
                nc.vector.tensor_copy(out=o16_t[:], in_=o2_t[:])

                nc.sync.dma_start(
                    out=rhs[:, g * GRP * 3:(g + 1) * GRP * 3], in_=o16_t[:])
    nc.compile()
    return nc


def host_stage(xyz1, neighborList, weightMatrix, rotations):
    """Build the fused [NCORES*BLOB_B] u8 upload blob (global, core-major)."""
    ngrp = NT // GRP
    ek = GRP * K
    p = np.ascontiguousarray(xyz1[0]).astype(np.float32, copy=False)
    r9 = rotations.reshape(N_FULL, 9).astype(np.float32, copy=False)
    table = np.zeros((NPAD, D), dtype=np.float16)
    table[:N_FULL, 0:3] = p
    table[:N_FULL, 3:12] = r9
    nbr = np.zeros((NPAD, K), dtype=np.int32)
    nbr[:N_FULL] = neighborList.reshape(N_FULL, K)
    w = np.zeros((NPAD, K), dtype=np.uint8)
    np.rint(weightMatrix.reshape(N_FULL, K) * 255.0, casting="unsafe",
            out=w[:N_FULL])
    blob = np.empty((NCORES, BLOB_B), dtype=np.uint8)
    for c in range(NCORES):
        sl = slice(c * SLOTS, (c + 1) * SLOTS)
        blob[c, 0:T_B] = table[sl].view(np.uint8).ravel()
        nb = nbr[sl].reshape(128, ngrp, ek)
        b0 = (nb & 255).astype(np.uint8)
        b1 = ((nb >> 8) & 255).astype(np.uint8)
        hi = (nb >> 16).astype(np.uint8)
        nib = hi[:, :, 0::2] | (hi[:, :, 1::2] << 4)
        blob[c, T_B:T_B + O_B] = np.concatenate(
            [b0, b1, nib], axis=2).ravel()
        blob[c, T_B + O_B:] = w[sl].ravel()
    return blob.ravel()


def _make_runner(nc):
    """Like bass2jax.run_bass_via_pjrt, but the jitted executable is built
    once and reused across kernel() calls (re-tracing + XLA re-compile per
    call costs ~1s against a 2s invocation)."""
    import jax
    from jax.experimental.shard_map import shard_map
    from jax.sharding import Mesh, PartitionSpec
    from concourse import bass2jax

    bass2jax.install_neuronx_cc_hook()
    partition_name = (nc.partition_id_tensor.name
                      if nc.partition_id_tensor else None)
    in_names, out_names, out_avals = [], [], []
    for alloc in nc.m.functions[0].allocations:
        if not isinstance(alloc, mybir.MemoryLocationSet):
            continue
        name = alloc.memorylocations[0].name
        if alloc.kind == "ExternalInput":
            if name != partition_name:
                in_names.append(name)
        elif alloc.kind == "ExternalOutput":
            out_names.append(name)
            out_avals.append(jax.core.ShapedArray(
                tuple(alloc.tensor_shape), mybir.dt.np(alloc.dtype)))
    assert in_names == ["blob"] and out_names == ["rhs"], (in_names, out_names)
    n_params = len(in_names)
    n_outs = len(out_avals)
    all_names = in_names + out_names
    if partition_name is not None:
        all_names.append(partition_name)

    def _body(*args):
        operands = list(args)
        if partition_name is not None:
            operands.append(bass2jax.partition_id_tensor())
        outs = bass2jax._bass_exec_p.bind(
            *operands,
            out_avals=tuple(out_avals),
            in_names=tuple(all_names),
            out_names=tuple(out_names),
            lowering_input_output_aliases=(),
            sim_require_finite=True,
            sim_require_nnan=True,
            nc=nc,
        )
        return tuple(outs)

    devices = jax.devices()[:NCORES]
    mesh = Mesh(np.asarray(devices), ("core",))
    in_specs = (PartitionSpec("core"),) * (n_params + n_outs)
    out_specs = (PartitionSpec("core"),) * n_outs
    sharded = jax.jit(
        shard_map(_body, mesh=mesh, in_specs=in_specs, out_specs=out_specs,
                  check_rep=False),
        donate_argnums=tuple(range(n_params, n_params + n_outs)),
        keep_unused=True,
    )

    def run(blob_all):
        # the kernel writes every rhs element; the donated buffer's
        # contents are irrelevant, so skip the host-side memset
        out_buf = np.empty((NCORES * 128, NT * 3), np.float16)
        out, = sharded(blob_all, out_buf)
        return np.asarray(out)

    return run


def kernel(xyz1, xyz2, neighborList, numNeighbors, accnumNeighbors,
           weightMatrix, rotations, arapWeight, trace=False):
    global LAST_EXEC_NS, LAST_RUN_WALL_S
    import time as _time
    xyz1 = np.asarray(xyz1)
    neighborList = np.asarray(neighborList)
    weightMatrix = np.asarray(weightMatrix)
    rotations = np.asarray(rotations)
    if "run" not in _CACHE:
        nc = build_kernel()
        _CACHE["run"] = _make_runner(nc)
    blob = host_stage(xyz1, neighborList, weightMatrix, rotations)
    _t0 = _time.time()
    rhs_all = _CACHE["run"](blob)
    LAST_RUN_WALL_S = _time.time() - _t0
    rhs_all = rhs_all.reshape(NCORES, 128, NT * 3)
    parts = []
    for c in range(NCORES):
        lo = c * SLOTS
        hi = min((c + 1) * SLOTS, N_FULL)
        parts.append(rhs_all[c].reshape(SLOTS, 3)[:hi - lo])
    return np.concatenate(parts, axis=0).astype(np.float32)
